# revision 1
# baseline (speedup 1.0000x reference)
"""Causal self-attention kernel for 8 Trainium2 NeuronCores.

Problem (hardcoded): x [4, 2048, 1024], torch-style Linear weights
W_q/W_k/W_v/W_o [1024, 1024], b_o [1024]; 16 heads, head_dim 64,
causal softmax attention, out = attn(x) @ W_o.T + b_o.

Sharding: 8 cores = 4 batches x 2 head-groups (8 heads each).
Each core computes a partial output  y_g @ W_o[:, g].T  for its batch;
the host sums the two head-group partials and adds b_o (unshard step).

Per-core pipeline (all matmuls on PE at 1 cycle/row):
  phase 1: QKV projections in float32r from xT [D, T] and pre-transposed
           weights; q/k written as qT/kT [dq, T] fp16, v as natural
           [T, dv] fp16 with an appended ones-column (v_aug).
  phase 2: per head: scores S^T[k, q] (K=64 matmuls) -> exp on ACT
           (scale=1/8) directly from PSUM into fp16 expP tiles (causal
           span only) -> causal masking via memset + triangular
           mask-multiply -> PV matmuls yT_aug[65, Tq] = v_aug.T @ expP;
           row 64 is the softmax denominator (ones-column trick);
           normalize yT via reciprocal + PE ones-broadcast + multiply.
  phase 3: out[T, D] = yT_norm.T @ W_o_g.T in fp16, fp32 out.
"""

import numpy as np

import concourse.bass as bass
import concourse.tile as tile
import concourse.mybir as mybir
from concourse import bacc
from concourse import bass_utils

T = 2048
D = 1024
HPC = 8            # heads per core
DH = 64
DQ = HPC * DH      # 512, per-core projection width
NT = T // 128      # 16 row tiles
NJ = DQ // 128     # 4 dq tiles
NC4 = T // 512     # 4 Tq chunks

F32 = mybir.dt.float32
F32R = mybir.dt.float32r
F16 = mybir.dt.float16
EXP = mybir.ActivationFunctionType.Exp

TRACE = False
LAST = None        # BassKernelResults of the most recent run

TRIMASK = np.triu(np.ones((128, 128), dtype=np.float16))


def _body(tc):
    nc = tc.nc
    xT_d = nc.dram_tensor("xt", (D, T), F32R, kind="ExternalInput").ap()
    wq_d = nc.dram_tensor("wqt", (D, DQ), F32R, kind="ExternalInput").ap()
    wk_d = nc.dram_tensor("wkt", (D, DQ), F32R, kind="ExternalInput").ap()
    # fp16 copy of x, host-laid as [p, t, k, col] for the v projection
    xv_d = nc.dram_tensor("xv", (128, NT, 8, 128), F16, kind="ExternalInput").ap()
    wv_d = nc.dram_tensor("wvt", (D, DQ), F16, kind="ExternalInput").ap()
    wo_d = nc.dram_tensor("wot", (DQ, D), F16, kind="ExternalInput").ap()
    tm_d = nc.dram_tensor("trimask", (128, 128), F16, kind="ExternalInput").ap()
    out_d = nc.dram_tensor("out", (T, D), F32, kind="ExternalOutput").ap()

    with (
        tc.tile_pool(name="persist", bufs=1) as pp,
        tc.tile_pool(name="psum_y", bufs=2, space="PSUM") as ypp,
    ):
        qT = pp.tile([128, NJ, T], F16, tag="qT")
        kT = pp.tile([128, NJ, T], F16, tag="kT")
        v = pp.tile([128, NT, HPC, DH + 1], F16, tag="v")
        yT = pp.tile([128, NJ, T], F16, tag="yT")
        woT = pp.tile([128, NJ, D], F16, tag="woT")
        trim = pp.tile([128, 128], F16, tag="trim")
        ones = pp.tile([1, DH], F16, tag="ones")

        nc.gpsimd.memset(ones[:], 1.0)
        nc.gpsimd.memset(v[:, :, :, DH:DH + 1], 1.0)
        # warm the ACT exp table while DMAs run
        warm = pp.tile([1, DH], F16, tag="warm")
        nc.scalar.activation(warm[:], ones[:], EXP, scale=1.0)

        # ---- phase 1a: Q/K projections (float32r) ----
        # DMA issue order matters: wq first so the first matmul can start
        # ~4us in; inputs needed later go last on the queue.
        with (
            tc.tile_pool(name="xpool", bufs=1) as xp,
            tc.tile_pool(name="wpool", bufs=2) as wp,
            tc.tile_pool(name="psum_q", bufs=6, space="PSUM") as qpp,
        ):
            wts = [wp.tile([128, 8, DQ], F32R, tag="w", name=f"w{i}")
                   for i in range(2)]
            xts = [xp.tile([128, T], F32R, tag=f"x{k}", name=f"xt_{k}")
                   for k in range(8)]

            def _wload(i, half):
                wsrc = (wq_d, wk_d)[i].rearrange("(c p) n -> p c n", p=128)
                nc.sync.dma_start(wts[i][:, 4 * half:4 * half + 4, :],
                                  wsrc[:, 4 * half:4 * half + 4, :])

            # split x loads across the HWDGE (sync) and SWDGE (gpsimd)
            # queues so they land in parallel
            _wload(0, 0)
            _wload(0, 1)
            for k in (1, 3, 5, 7):
                nc.gpsimd.dma_start(xts[k][:], xT_d[128 * k:128 * (k + 1), :])
            for k in (0, 2, 4, 6):
                nc.sync.dma_start(xts[k][:], xT_d[128 * k:128 * (k + 1), :])
            _wload(1, 0)
            _wload(1, 1)
            nc.sync.dma_start(trim[:], tm_d)

            # contraction in approximate DMA-arrival order
            KS = (1, 3, 0, 5, 2, 7, 4, 6)
            for wt, dest in ((wts[0], qT), (wts[1], kT)):
                for j in range(NJ):
                    for c in range(NC4):
                        ps = qpp.tile([128, 512], F32, tag="q")
                        for ki, k in enumerate(KS):
                            nc.tensor.matmul(
                                ps[:],
                                wt[:, k, 128 * j:128 * (j + 1)],
                                xts[k][:, 512 * c:512 * (c + 1)],
                                start=(ki == 0), stop=(ki == 7),
                            )
                        nc.vector.tensor_copy(dest[:, j, 512 * c:512 * (c + 1)], ps[:])

        # ---- phases 1b+2+3: v-projection (fp16) + attention + out-proj ----
        with (
            tc.tile_pool(name="xvpool", bufs=4) as xvp,
            tc.tile_pool(name="wvpool", bufs=1) as wvp,
            tc.tile_pool(name="psum_st", bufs=2, space="PSUM") as stp,
            tc.tile_pool(name="psum_g", bufs=2, space="PSUM") as gpp,
            tc.tile_pool(name="expp", bufs=2) as epool,
            tc.tile_pool(name="small", bufs=2) as sp,
            tc.tile_pool(name="outsb", bufs=3) as op,
        ):
            wv16 = wvp.tile([128, 8, DQ], F16, tag="wv")
            wvsrc = wv_d.rearrange("(c p) n -> p c n", p=128)
            nc.sync.dma_start(wv16[:], wvsrc)
            xv_tiles = [xvp.tile([128, 8, 128], F16, tag="xv", name=f"xv_{t}")
                        for t in range(NT)]
            for t in range(NT):
                nc.sync.dma_start(xv_tiles[t][:], xv_d[:, t, :, :])
            nc.sync.dma_start(woT[:], wo_d.rearrange("(j p) n -> p j n", p=128))

            def scores_head(h):
                hp = (h % 2) * DH
                hj = h // 2
                expps = [None] * NT
                for r in range(NT):
                    s0 = 512 * (r // 4)
                    ep = epool.tile([128, T - s0], F16, tag=f"e{r}",
                                    name=f"e{r}_h{h}", bufs=2)
                    expps[r] = (ep, s0)
                    if 128 * r > s0:
                        nc.gpsimd.memset(ep[:, 0:128 * r - s0], 0.0)
                    bounds = ([(s0, 1024), (1024, 2048)] if s0 < 1024
                              else [(s0, 2048)])
                    for (lo, hi) in bounds:
                        st = stp.tile([128, hi - lo], F32, tag="st")
                        for n0 in range(lo, hi, 512):
                            c0 = max(n0, 128 * r)  # exact-causal start
                            nc.tensor.matmul(
                                st[:, c0 - lo:n0 - lo + 512],
                                kT[hp:hp + DH, hj, 128 * r:128 * (r + 1)],
                                qT[hp:hp + DH, hj, c0:n0 + 512],
                                start=True, stop=True,
                            )
                        elo = max(lo, 128 * r)
                        nc.scalar.activation(
                            ep[:, elo - s0:hi - s0], st[:, elo - lo:hi - lo],
                            EXP, scale=0.125,
                        )
                    db = 128 * r - s0
                    nc.gpsimd.tensor_mul(
                        ep[:, db:db + 128], ep[:, db:db + 128], trim[:]
                    )
                return expps

            def pv_head(h, expps, corder=None):
                hp = (h % 2) * DH
                hj = h // 2
                for c in (corder or range(NC4)):
                    ya = ypp.tile([DH + 1, 512], F32, tag="y")
                    rmax = 4 * c + 3
                    for r in range(rmax + 1):
                        ep, s0 = expps[r]
                        off = max(0, 128 * r - 512 * c)  # exact-causal start
                        nc.tensor.matmul(
                            ya[:, off:512], v[:, r, h, :],
                            ep[:, 512 * c - s0 + off:512 * c - s0 + 512],
                            start=(r == 0), stop=(r == rmax),
                        )
                    yt = sp.tile([DH + 1, 512], F16, tag="yt")
                    nc.vector.tensor_copy(yt[:], ya[:])
                    rec = sp.tile([1, 512], F16, tag="rec")
                    with nc.allow_low_precision(reason="softmax reciprocal fp16"):
                        nc.vector.reciprocal(rec[:], ya[DH:DH + 1, :])
                    bc = ypp.tile([DH, 512], F32, tag="y")
                    nc.tensor.matmul(bc[:], ones[:], rec[:], start=True, stop=True)
                    bcs = sp.tile([DH, 512], F16, tag="bcs")
                    nc.vector.tensor_copy(bcs[:], bc[:])
                    nc.gpsimd.tensor_mul(
                        yT[hp:hp + DH, hj, 512 * c:512 * (c + 1)], yt[0:DH, :], bcs[:]
                    )

            def vproj(t0, t1):
                for t in range(t0, t1):
                    ps = gpp.tile([128, 512], F32, tag="g")
                    for k in range(8):
                        nc.tensor.matmul(
                            ps[:],
                            xv_tiles[t][:, k, :],
                            wv16[:, k, :],
                            start=(k == 0), stop=(k == 7),
                        )
                    nc.vector.tensor_copy(
                        v[:, t, :, 0:DH], ps[:].rearrange("p (h d) -> p h d", h=HPC)
                    )

            def outproj_tile(i):
                ob = op.tile([128, D], F32, tag="ob", name=f"ob{i}")
                for d in range(2):
                    ps = gpp.tile([128, 512], F32, tag="g", name=f"op{i}_{d}")
                    for j in range(NJ):
                        nc.tensor.matmul(
                            ps[:],
                            yT[:, j, 128 * i:128 * (i + 1)],
                            woT[:, j, 512 * d:512 * (d + 1)],
                            start=(j == 0), stop=(j == 3),
                        )
                    nc.scalar.copy(ob[:, 512 * d:512 * (d + 1)], ps[:])
                nc.sync.dma_start(out_d[128 * i:128 * (i + 1), :], ob[:])

            # Software-pipelined heads: scores for h0+h1 first so ACT gets
            # continuous work, fp16 v-projection splits around them on PE,
            # then scores(h+1) is emitted ahead of pv(h) throughout.
            expps = {0: scores_head(0)}
            vproj(0, 9)
            expps[1] = scores_head(1)
            vproj(9, NT)
            for h in range(HPC):
                if h + 2 < HPC:
                    expps[h + 2] = scores_head(h + 2)
                pv_head(h, expps.pop(h))

            # ---- phase 3: output projection (fp16) ----
            for i in range(NT):
                outproj_tile(i)



def build_nc():
    nc = bacc.Bacc("TRN2", target_bir_lowering=False, debug=False)
    with tile.TileContext(nc) as tc:
        _body(tc)
    nc.compile()
    return nc


_nc_cache = None


def _get_nc():
    global _nc_cache
    if _nc_cache is None:
        _nc_cache = build_nc()
    return _nc_cache


def make_in_maps(x, W_q, W_k, W_v, W_o):
    x = np.asarray(x, dtype=np.float32)
    W_q = np.asarray(W_q, dtype=np.float32)
    W_k = np.asarray(W_k, dtype=np.float32)
    W_v = np.asarray(W_v, dtype=np.float32)
    W_o = np.asarray(W_o, dtype=np.float32)
    in_maps = []
    for c in range(8):
        b, g = divmod(c, 2)
        sl = slice(DQ * g, DQ * (g + 1))
        xT = np.ascontiguousarray(x[b].T)
        # [p, t, k, col] layout for the fp16 v-projection streaming tiles
        xv = np.ascontiguousarray(
            xT.astype(np.float16).reshape(8, 128, NT, 128).transpose(1, 2, 0, 3)
        )
        in_maps.append({
            "xt": xT,
            "xv": xv,
            "wqt": np.ascontiguousarray(W_q[sl].T),
            "wkt": np.ascontiguousarray(W_k[sl].T),
            "wvt": np.ascontiguousarray(W_v[sl].T).astype(np.float16),
            "wot": np.ascontiguousarray(W_o[:, sl].T).astype(np.float16),
            "trimask": TRIMASK,
        })
    return in_maps


def kernel(x, W_q, W_k, W_v, W_o, b_o):
    global LAST
    nc = _get_nc()
    in_maps = make_in_maps(x, W_q, W_k, W_v, W_o)
    res = bass_utils.run_bass_kernel_spmd(
        nc, in_maps, core_ids=list(range(8)), trace=TRACE
    )
    LAST = res
    parts = [res.results[c]["out"] for c in range(8)]
    b_o = np.asarray(b_o, dtype=np.float32)
    out = np.stack([parts[2 * b] + parts[2 * b + 1] for b in range(4)])
    out += b_o[None, None, :]
    return out.astype(np.float32)



# revision 37
# speedup vs baseline: 1.2944x; 1.2944x over previous
"""Causal self-attention kernel for 8 Trainium2 NeuronCores.

Problem (hardcoded): x [4, 2048, 1024], torch-style Linear weights
W_q/W_k/W_v/W_o [1024, 1024], b_o [1024]; 16 heads, head_dim 64,
causal softmax attention, out = attn(x) @ W_o.T + b_o.

Sharding: 8 cores = 4 batches x 2 head-groups (8 heads each).
Each core computes a partial output  y_g @ W_o[:, g].T  for its batch;
the host sums the two head-group partials and adds b_o (unshard step).

Per-core pipeline (all matmuls fp16 on PE at 1 cycle/row):
  - QKV projections from an fp16 xT tile layout [p, tt, k, t]; Q/K written
    per-j-pair as qT/kT [128(2 heads x 64d), T]; V natural [k, h, 65] with
    an appended ones-column (denominator trick).
  - scores per head: S^T[k, q] over the exact causal span, exp on ACT
    (scale 1/8) into fp16 ep tiles [128k, 2048-128r]; diagonal block
    masked by a triangular mask-multiply on gpsimd.
  - PV swapped: stationary ep block [128k, 128q], moving v_aug [128k, 65]
    -> ya[q, 65] in PSUM (col 64 = softmax denominator).  Normalize with
    DVE reciprocal + per-partition tensor_scalar_mul into yn2 [q, 128]
    (two heads side by side), then DMA-transpose into yT [128(pair d), T].
  - out[T, D] = yT.T @ W_o in fp16 halves, fp32 out, DMA per half.
  - Software-pipelined: warmup matmuls ramp the PE clock, QK j-tiles /
    vproj / scores / pv are interleaved so PE never starves while ACT
    grinds through the exps; tail interleaves pv(6)/pv(7)/outproj.
"""

import numpy as np

import concourse.bass as bass
import concourse.tile as tile
import concourse.mybir as mybir
from concourse import bacc
from concourse import bass_utils

T = 2048
D = 1024
HPC = 8            # heads per core
DH = 64
DQ = HPC * DH      # 512, per-core projection width
NT = T // 128      # 16 row tiles
NJ = DQ // 128     # 4 head-pair tiles

F32 = mybir.dt.float32
F16 = mybir.dt.float16
EXP = mybir.ActivationFunctionType.Exp

TRACE = False
LAST = None        # BassKernelResults of the most recent run

TRIMASK = np.triu(np.ones((128, 128), dtype=np.float16))


def _body(tc):
    nc = tc.nc
    # fp16 copy of x^T, host-laid as [p, tt, k, t] tiles
    xv_d = nc.dram_tensor("xv", (128, NT, 8, 128), F16, kind="ExternalInput").ap()
    wq_d = nc.dram_tensor("wqt", (128, 8, DQ), F16, kind="ExternalInput").ap()
    wk_d = nc.dram_tensor("wkt", (128, 8, DQ), F16, kind="ExternalInput").ap()
    wv_d = nc.dram_tensor("wvt", (128, 8, DQ), F16, kind="ExternalInput").ap()
    wo_d = nc.dram_tensor("wot", (128, NJ, D), F16, kind="ExternalInput").ap()
    tm_d = nc.dram_tensor("trimask", (128, 128), F16, kind="ExternalInput").ap()
    out_d = nc.dram_tensor("out", (T, D), F32, kind="ExternalOutput").ap()

    with (
        tc.tile_pool(name="persist", bufs=1) as pp,
        tc.tile_pool(name="qk_sb", bufs=2) as qkp,
        tc.tile_pool(name="expp", bufs=2) as epool,
        tc.tile_pool(name="yn2p", bufs=1) as ynp,
        tc.tile_pool(name="outsb", bufs=4) as op,
        tc.tile_pool(name="small", bufs=2) as sp,
    ):
        xv = pp.tile([128, NT, 8, 128], F16, tag="xv")
        wq16 = pp.tile([128, 8, DQ], F16, tag="wq16")
        wk16 = pp.tile([128, 8, DQ], F16, tag="wk16")
        wv16 = pp.tile([128, 8, DQ], F16, tag="wv16")
        woT = pp.tile([128, NJ, D], F16, tag="woT")
        v = pp.tile([128, NT, HPC, DH + 1], F16, tag="v")
        yT = pp.tile([128, NJ, T], F16, tag="yT")
        trim = pp.tile([128, 128], F16, tag="trim")
        warm = pp.tile([1, DH], F16, tag="warm")
        dumA = pp.tile([128, 128], F16, tag="dumA")
        dumB = pp.tile([128, 512], F16, tag="dumB")
        qTs = {}
        kTs = {}
        yn2 = [ynp.tile([128, 128], F16, tag=f"yn2_{i}", name=f"yn2_{i}")
               for i in range(NT)]

        # ---- t~0: memsets + ACT exp-table warm ----
        nc.vector.memset(dumA[:], 0.001)
        nc.vector.memset(dumB[:], 0.001)
        nc.gpsimd.memset(v[:, :, :, DH:DH + 1], 1.0)
        nc.scalar.activation(warm[:], dumA[0:1, 0:DH], EXP, scale=1.0)

        # ---- DMA issue: 3 queues in parallel, startup-critical first ----
        # sync: xv0 + even wq chunks + xv 2,4,8,12; scalar: xv1 + odd wq
        # chunks + wk + xv 3,5,9,13; gpsimd(SWDGE, behind the memsets):
        # xv 6,7,10,11,14,15 + wv + trim + woT
        nc.sync.dma_start(xv[:, 0, :, :], xv_d[:, 0, :, :])
        nc.scalar.dma_start(xv[:, 1, :, :], xv_d[:, 1, :, :])
        for k in range(0, 4):
            eng = nc.sync if k % 2 == 0 else nc.scalar
            eng.dma_start(wq16[:, k:k + 1, :], wq_d[:, k:k + 1, :])
        nc.scalar.dma_start(xv[:, 3, :, :], xv_d[:, 3, :, :])
        nc.sync.dma_start(xv[:, 2, :, :], xv_d[:, 2, :, :])
        for k in range(4, 8):
            eng = nc.sync if k % 2 == 0 else nc.scalar
            eng.dma_start(wq16[:, k:k + 1, :], wq_d[:, k:k + 1, :])
        nc.sync.dma_start(xv[:, 4, :, :], xv_d[:, 4, :, :])
        nc.scalar.dma_start(xv[:, 5, :, :], xv_d[:, 5, :, :])
        for t in (6, 7):
            nc.gpsimd.dma_start(xv[:, t, :, :], xv_d[:, t, :, :])
        nc.sync.dma_start(xv[:, 8, :, :], xv_d[:, 8, :, :])
        nc.scalar.dma_start(wk16[:, 0:2, :], wk_d[:, 0:2, :])
        nc.scalar.dma_start(xv[:, 9, :, :], xv_d[:, 9, :, :])
        for t in (10, 11):
            nc.gpsimd.dma_start(xv[:, t, :, :], xv_d[:, t, :, :])
        nc.scalar.dma_start(wk16[:, 2:4, :], wk_d[:, 2:4, :])
        nc.sync.dma_start(xv[:, 12, :, :], xv_d[:, 12, :, :])
        nc.scalar.dma_start(xv[:, 13, :, :], xv_d[:, 13, :, :])
        for t in (14, 15):
            nc.gpsimd.dma_start(xv[:, t, :, :], xv_d[:, t, :, :])
        for k in range(4, 8, 2):
            nc.scalar.dma_start(wk16[:, k:k + 2, :], wk_d[:, k:k + 2, :])
        nc.gpsimd.dma_start(wv16[:], wv_d)
        nc.gpsimd.dma_start(trim[:], tm_d)
        nc.gpsimd.dma_start(woT[:], wo_d)

        with (
            tc.tile_pool(name="psum_dum", bufs=1, space="PSUM") as dpp,
        ):
            # PE clock warmup while the first DMAs land (~11 x 512 rows)
            dps = dpp.tile([128, 512], F32, tag="dum")
            for _ in range(11):
                nc.tensor.matmul(dps[:], dumA[:], dumB[:], start=True, stop=True)

        with (
            tc.tile_pool(name="psum_g", bufs=2, space="PSUM") as gpp,
            tc.tile_pool(name="psum_y", bufs=2, space="PSUM") as yap,
        ):
            stp_box = [None]
            def qk_g(w, j, g, fine=False):
                """One [128, 512] projection group of Q or K for pair j."""
                wt = wq16 if w == "q" else wk16
                store = qTs if w == "q" else kTs
                if j not in store:
                    store[j] = qkp.tile([128, T], F16, tag=f"{w}T",
                                        name=f"{w}T{j}")
                dst = store[j]
                ps = gpp.tile([128, 512], F32, tag="g", name=f"{w}p{j}_{g}")
                if fine:
                    # per-tt sub-groups so the first matmuls only need one
                    # xv tile + one weight chunk (startup DMA pacing)
                    for tt in range(4):
                        for k in range(8):
                            nc.tensor.matmul(
                                ps[:, 128 * tt:128 * (tt + 1)],
                                wt[:, k, 128 * j:128 * (j + 1)],
                                xv[:, 4 * g + tt, k, :],
                                start=(k == 0), stop=(k == 7),
                            )
                else:
                    for k in range(8):
                        nc.tensor.matmul(
                            ps[:],
                            wt[:, k, 128 * j:128 * (j + 1)],
                            xv[:, 4 * g:4 * (g + 1), k, :],
                            start=(k == 0), stop=(k == 7),
                        )
                nc.vector.tensor_copy(dst[:, 512 * g:512 * (g + 1)], ps[:])

            def vp(t, half):
                """V projection of key-tile t for heads [4*half, 4*half+4)."""
                ps = gpp.tile([128, 256], F32, tag="g", name=f"vp{t}_{half}")
                for k in range(8):
                    nc.tensor.matmul(
                        ps[:],
                        xv[:, t, k, :],
                        wv16[:, k, 256 * half:256 * (half + 1)],
                        start=(k == 0), stop=(k == 7),
                    )
                nc.vector.tensor_copy(
                    v[:, t, 4 * half:4 * (half + 1), 0:DH],
                    ps[:].rearrange("p (h d) -> p h d", h=4))

            eps = {h: [None] * NT for h in range(HPC)}

            def score_r(h, r):
                """Key-tile r of head h: S^T matmuls + exp + diagonal mask."""
                hp = (h % 2) * DH
                hj = h // 2
                qT, kT = qTs[hj], kTs[hj]
                lo0 = 128 * r
                ep = epool.tile([128, T - lo0], F16, tag=f"e{r}",
                                name=f"e{r}_h{h}")
                eps[h][r] = ep
                # st tiles start on 512-aligned bases so every matmul
                # chunk stays inside one PSUM bank
                a0 = 512 * (r // 4)
                bounds = ([(a0, 1024), (1024, 2048)] if lo0 < 1024
                          else [(a0, 2048)])
                for (lo, hi) in bounds:
                    st = stp_box[0].tile([128, hi - lo], F32, tag="st")
                    elo = max(lo, lo0)
                    for n0 in range(lo, hi, 512):
                        c0 = max(n0, lo0)
                        nc.tensor.matmul(
                            st[:, c0 - lo:n0 - lo + 512],
                            kT[hp:hp + DH, lo0:lo0 + 128],
                            qT[hp:hp + DH, c0:n0 + 512],
                            start=True, stop=True,
                        )
                    nc.scalar.activation(
                        ep[:, elo - lo0:hi - lo0], st[:, elo - lo:hi - lo],
                        EXP, scale=0.125,
                    )
                nc.gpsimd.tensor_mul(ep[:, 0:128], ep[:, 0:128], trim[:])

            def pv_t(h, i):
                """One q-tile of swapped PV + normalize into yn2[i]."""
                hp = (h % 2) * DH
                hj = h // 2
                ya = yap.tile([128, DH + 1], F32, tag="y", name=f"ya{h}_{i}")
                for r in range(i + 1):
                    off = 128 * (i - r)
                    nc.tensor.matmul(
                        ya[:], eps[h][r][:, off:off + 128], v[:, r, h, :],
                        start=(r == 0), stop=(r == i),
                    )
                rec = sp.tile([128, 1], F32, tag="rec", name=f"rc{h}_{i}")
                nc.vector.reciprocal(rec[:], ya[:, DH:DH + 1])
                nc.vector.tensor_scalar_mul(
                    yn2[i][:, hp:hp + DH], ya[:, 0:DH], rec[:])
                if h % 2 == 1:
                    nc.sync.dma_start_transpose(
                        yT[:, hj, 128 * i:128 * (i + 1)], yn2[i][:])

            def op_h(i, d, pool, split=1):
                """Half of output-projection row tile i, copy + DMA out."""
                ob = op.tile([128, 512], F32, tag="ob", name=f"ob{i}_{d}")
                ps = pool.tile([128, 512], F32, tag="opp", name=f"opp{i}_{d}")
                for j in range(NJ):
                    nc.tensor.matmul(
                        ps[:],
                        yT[:, j, 128 * i:128 * (i + 1)],
                        woT[:, j, 512 * d:512 * (d + 1)],
                        start=(j == 0), stop=(j == 3),
                    )
                w = 512 // split
                for q in range(split):
                    if d == 0:
                        nc.vector.tensor_copy(ob[:, q * w:(q + 1) * w],
                                              ps[:, q * w:(q + 1) * w])
                    else:
                        nc.scalar.copy(ob[:, q * w:(q + 1) * w],
                                       ps[:, q * w:(q + 1) * w])
                    nc.sync.dma_start(
                        out_d[128 * i:128 * (i + 1),
                              512 * d + q * w:512 * d + (q + 1) * w],
                        ob[:, q * w:(q + 1) * w])

            # ---- the interleaved schedule (PE program order) ----
            # Regions: during score_r(h, .) slots, filler units keep PE dense
            # while ACT grinds exp(h); pv(h-1) lags 2 key-tiles behind.
            stp_ctx = tc.tile_pool(name="psum_st", bufs=2, space="PSUM")
            stp_box[0] = stp_ctx.__enter__()
            for g in range(4):                       # Q of pair 0
                qk_g("q", 0, g)
            # region 0: K j0 + scores(0) + vproj half0 tt 0..7
            for r in range(NT):
                if r % 4 == 0:
                    qk_g("k", 0, r // 4)
                score_r(0, r)
                if r < 8:
                    vp(r, 0)
            # region 1: scores(1) + Q j1 + vproj half0 tt 8..15 + pv(0)
            for r in range(NT):
                score_r(1, r)
                if r < 4:
                    vp(8 + r, 0)
                elif r < 8:
                    qk_g("q", 1, r - 4)
                elif r < 12:
                    vp(4 + r, 0)
                if r >= 2:
                    pv_t(0, r - 2)
            pv_t(0, 14)
            pv_t(0, 15)
            # region 2: K j1 + scores(2) + Q j2 + pv(1)
            for r in range(NT):
                if r % 4 == 0:
                    qk_g("k", 1, r // 4)
                score_r(2, r)
                if 8 <= r < 12:
                    qk_g("q", 2, r - 8)
                if r >= 2:
                    pv_t(1, r - 2)
            pv_t(1, 14)
            pv_t(1, 15)
            # region 3: K j2 + scores(3) + pv(2)
            for r in range(NT):
                if r % 4 == 0:
                    qk_g("k", 2, r // 4)
                score_r(3, r)
                if r >= 2:
                    pv_t(2, r - 2)
            pv_t(2, 14)
            pv_t(2, 15)
            # region 4: scores(4) + vproj half1 tt 0..7 + Q j3 + pv(3)
            for r in range(NT):
                score_r(4, r)
                if 4 <= r < 12:
                    vp(r - 4, 1)
                if r >= 12:
                    qk_g("q", 3, r - 12)
                if r >= 2:
                    pv_t(3, r - 2)
            pv_t(3, 14)
            pv_t(3, 15)
            # region 5: scores(5) + vproj half1 tt 8..15 + pv(4)
            for r in range(NT):
                score_r(5, r)
                if 2 <= r < 10:
                    vp(r + 6, 1)
                if r >= 2:
                    pv_t(4, r - 2)
            pv_t(4, 14)
            pv_t(4, 15)
            # region 6: K j3 + scores(6) + pv(5)
            for r in range(NT):
                if r % 4 == 0:
                    qk_g("k", 3, r // 4)
                score_r(6, r)
                if r >= 2:
                    pv_t(5, r - 2)
            pv_t(5, 14)
            pv_t(5, 15)
            # region 7: scores(7) + pv(6)
            for r in range(NT):
                score_r(7, r)
                if r >= 2:
                    pv_t(6, r - 2)
            pv_t(6, 14)
            pv_t(6, 15)
            stp_ctx.__exit__(None, None, None)
            # tail: pv(7) q-tiles interleaved with the output projection;
            # the freed score-PSUM banks become outproj accumulators
            with tc.tile_pool(name="psum_op", bufs=4, space="PSUM") as opp:
                for i in range(NT - 2):
                    pv_t(7, i)
                    op_h(i, 0, opp)
                    op_h(i, 1, opp)
                # last two q-tiles: both pv's first so op(15) does not wait
                # on its transpose
                pv_t(7, NT - 2)
                pv_t(7, NT - 1)
                op_h(NT - 2, 0, opp)
                op_h(NT - 2, 1, opp)
                op_h(NT - 1, 0, opp)
                op_h(NT - 1, 1, opp)


def build_nc():
    nc = bacc.Bacc("TRN2", target_bir_lowering=False, debug=False)
    with tile.TileContext(nc) as tc:
        _body(tc)
    nc.compile()
    return nc


_nc_cache = None


def _get_nc():
    global _nc_cache
    if _nc_cache is None:
        _nc_cache = build_nc()
    return _nc_cache


def make_in_maps(x, W_q, W_k, W_v, W_o):
    x = np.asarray(x, dtype=np.float32)
    W_q = np.asarray(W_q, dtype=np.float32)
    W_k = np.asarray(W_k, dtype=np.float32)
    W_v = np.asarray(W_v, dtype=np.float32)
    W_o = np.asarray(W_o, dtype=np.float32)

    def wlay(w):   # [512, 1024] (torch [out,in] slice) -> [128, 8, 512] fp16
        return np.ascontiguousarray(
            w.T.reshape(8, 128, DQ).transpose(1, 0, 2)).astype(np.float16)

    in_maps = []
    for c in range(8):
        b, g = divmod(c, 2)
        sl = slice(DQ * g, DQ * (g + 1))
        xT = np.ascontiguousarray(x[b].T).astype(np.float16)
        # [p, tt, k, t] layout for the fp16 x^T streaming tiles
        xv = np.ascontiguousarray(
            xT.reshape(8, 128, NT, 128).transpose(1, 2, 0, 3))
        wo = np.ascontiguousarray(
            W_o[:, sl].T.reshape(NJ, 128, D).transpose(1, 0, 2)
        ).astype(np.float16)
        in_maps.append({
            "xv": xv,
            "wqt": wlay(W_q[sl]),
            "wkt": wlay(W_k[sl]),
            "wvt": wlay(W_v[sl]),
            "wot": wo,
            "trimask": TRIMASK,
        })
    return in_maps


def kernel(x, W_q, W_k, W_v, W_o, b_o):
    global LAST
    nc = _get_nc()
    in_maps = make_in_maps(x, W_q, W_k, W_v, W_o)
    res = bass_utils.run_bass_kernel_spmd(
        nc, in_maps, core_ids=list(range(8)), trace=TRACE
    )
    LAST = res
    parts = [res.results[c]["out"] for c in range(8)]
    b_o = np.asarray(b_o, dtype=np.float32)
    out = np.stack([parts[2 * b] + parts[2 * b + 1] for b in range(4)])
    out += b_o[None, None, :]
    return out.astype(np.float32)
